# revision 1
# baseline (speedup 1.0000x reference)
"""Trainium2 Bass kernel for GQA attention (RoPE + QK-L2norm + causal + wo).

Full-problem shapes: B=2, T=2048, DIM=2048, H=32 q-heads, G=8 kv-heads, D=64.
Sharding over 8 cores: core = (batch b = c//4, kv-pair p = c%4) ->
8 q heads / 2 kv heads per core. Each core computes a partial (feature-major)
output out_T = wo_slice @ y_slice of shape [DIM, T]; host sums 4 partials per
batch and transposes.

Per-core layouts (feature-major): xT [DIM, T], q/k projected as [feat, T].
RoPE trick: weight rows permuted per head so rotation pairs sit at partition
offset +-16 inside each 32-block -> stream_shuffle provides the rotated
operand at full 128-lane width.
"""
import math
from contextlib import ExitStack

import numpy as np
import ml_dtypes

import concourse.bass as bass
import concourse.tile as tile
from concourse import mybir

F32 = mybir.dt.float32
BF16 = mybir.dt.bfloat16

NEG = -30000.0  # additive causal mask value (exp -> 0)


# ---------------------------------------------------------------- host prep
def _head_perm(D=64):
    """Permutation: new position j (0..63) -> original dim index."""
    perm = np.empty(D, dtype=np.int64)
    for j in range(D):
        block, q = j // 32, j % 32
        freq = (q % 16) + 16 * block
        perm[j] = 2 * freq + (1 if q >= 16 else 0)
    return perm


def _freq_of_partition(P=128):
    fr = np.empty(P, dtype=np.int64)
    sign = np.empty(P, dtype=np.float32)
    for p in range(P):
        fr[p] = (p % 16) + 16 * ((p % 64) // 32)
        sign[p] = -1.0 if (p % 32) < 16 else 1.0
    return fr, sign


def prep_core_inputs(x, freqs_cos, freqs_sin, wq, wk, wv, wo, q_scale, k_scale,
                     T=2048, DIM=2048, D=64):
    """Returns list of 8 in_maps (numpy arrays, keyed by dram tensor names)."""
    bf = ml_dtypes.bfloat16
    perm = _head_perm(D)
    fr, sign = _freq_of_partition(128)

    C = freqs_cos[:, fr].T.astype(np.float32).copy()          # [128, T]
    Sp = (sign[:, None] * freqs_sin[:, fr].T).astype(np.float32).copy()

    r, c = np.mgrid[0:128, 0:128]
    maskT = np.where(c >= r, 0.0, NEG).astype(np.float32)
    mask01 = np.where(c >= r, 1.0, 0.0).astype(bf)

    selq = np.zeros((128, 4, 8), np.float32)
    for m in range(4):
        selq[0:64, m, 2 * m] = 1.0
        selq[64:128, m, 2 * m + 1] = 1.0
    selq = selq.reshape(128, 32).astype(bf).copy()
    selk = np.zeros((128, 2), bf)
    selk[0:64, 0] = 1.0
    selk[64:128, 1] = 1.0
    sel2q = np.zeros((8, 4, 128), np.float32)
    for m in range(4):
        sel2q[2 * m, m, 0:64] = 1.0
        sel2q[2 * m + 1, m, 64:128] = 1.0
    sel2q = sel2q.reshape(8, 512).astype(bf).copy()
    sel2k = np.zeros((2, 128), bf)
    sel2k[0, 0:64] = 1.0
    sel2k[1, 64:128] = 1.0
    ident = np.eye(128, dtype=np.float32).astype(bf)

    qs = np.asarray(q_scale).reshape(-1)   # [32]
    ks = np.asarray(k_scale).reshape(-1)   # [8]

    xT = {b: np.ascontiguousarray(x[b].T).astype(bf) for b in range(x.shape[0])}

    in_maps = []
    for core in range(8):
        b, p = core // 4, core % 4
        # wq rows for this core, permuted per head, transposed
        wq_rows = wq[512 * p: 512 * (p + 1), :].reshape(8, D, DIM)[:, perm, :]
        wqT = np.ascontiguousarray(wq_rows.reshape(512, DIM).T).astype(bf)
        wk_rows = wk[128 * p: 128 * (p + 1), :].reshape(2, D, DIM)[:, perm, :]
        wkT = np.ascontiguousarray(wk_rows.reshape(128, DIM).T).astype(bf)
        wvT = np.ascontiguousarray(wv[128 * p: 128 * (p + 1), :].T).astype(bf)
        woT = np.ascontiguousarray(wo[:, 512 * p: 512 * (p + 1)].T).astype(bf)

        fh = np.empty((8, 1), np.float32)
        for h in range(8):
            fh[h, 0] = (qs[8 * p + h] * ks[2 * p + h // 4] / math.sqrt(D)) ** 2

        in_maps.append({
            "xT": xT[b], "wqT": wqT, "wkT": wkT, "wvT": wvT, "woT": woT,
            "Ct": C, "Sp": Sp, "maskT": maskT, "mask01": mask01, "fsq": fh, "ident": ident,
            "selq": selq, "selk": selk, "sel2q": sel2q, "sel2k": sel2k,
        })
    return in_maps


# ---------------------------------------------------------------- device code
def build_nc(T=2048, DIM=2048):
    """Build the per-core Bass program (identical for all 8 cores)."""
    NT = T // 512          # number of 512-wide token chunks
    ND = DIM // 128        # contraction tiles
    NTT = T // 128         # token tiles (tk)
    nc = bass.Bass("TRN2", target_bir_lowering=False, debug=False)

    xT = nc.dram_tensor("xT", [DIM, T], BF16, kind="ExternalInput").ap()
    wqT = nc.dram_tensor("wqT", [DIM, 512], BF16, kind="ExternalInput").ap()
    wkT = nc.dram_tensor("wkT", [DIM, 128], BF16, kind="ExternalInput").ap()
    wvT = nc.dram_tensor("wvT", [DIM, 128], BF16, kind="ExternalInput").ap()
    woT = nc.dram_tensor("woT", [512, DIM], BF16, kind="ExternalInput").ap()
    Ct = nc.dram_tensor("Ct", [128, T], F32, kind="ExternalInput").ap()
    Sp = nc.dram_tensor("Sp", [128, T], F32, kind="ExternalInput").ap()
    maskT = nc.dram_tensor("maskT", [128, 128], F32, kind="ExternalInput").ap()
    mask01 = nc.dram_tensor("mask01", [128, 128], BF16, kind="ExternalInput").ap()
    fsq = nc.dram_tensor("fsq", [8, 1], F32, kind="ExternalInput").ap()
    selq = nc.dram_tensor("selq", [128, 32], BF16, kind="ExternalInput").ap()
    selk = nc.dram_tensor("selk", [128, 2], BF16, kind="ExternalInput").ap()
    sel2q = nc.dram_tensor("sel2q", [8, 512], BF16, kind="ExternalInput").ap()
    sel2k = nc.dram_tensor("sel2k", [2, 128], BF16, kind="ExternalInput").ap()
    ident = nc.dram_tensor("ident", [128, 128], BF16, kind="ExternalInput").ap()
    outT = nc.dram_tensor("outT", [DIM, T], F32, kind="ExternalOutput").ap()

    SHUF = [(i + 16) % 32 for i in range(32)]
    EXP = mybir.ActivationFunctionType.Exp
    SQUARE = mybir.ActivationFunctionType.Square
    SQRT = mybir.ActivationFunctionType.Sqrt
    MUL = mybir.AluOpType.mult
    ADD = mybir.AluOpType.add

    with tile.TileContext(nc) as tc, ExitStack() as ctx:
        const = ctx.enter_context(tc.tile_pool(name="const", bufs=1))
        persist = ctx.enter_context(tc.tile_pool(name="persist", bufs=1))

        c_mask = const.tile([128, 128], F32)
        nc.sync.dma_start(out=c_mask[:], in_=maskT)
        c_m01 = const.tile([128, 128], BF16)
        nc.sync.dma_start(out=c_m01[:], in_=mask01)
        c_fsq = const.tile([8, 1], F32)
        nc.sync.dma_start(out=c_fsq[:], in_=fsq)
        c_selq = const.tile([128, 4, 8], BF16)
        nc.sync.dma_start(out=c_selq[:], in_=selq.rearrange("p (m h) -> p m h", m=4))
        c_selk = const.tile([128, 2], BF16)
        nc.sync.dma_start(out=c_selk[:], in_=selk)
        c_sel2q = const.tile([8, 4, 128], BF16)
        nc.sync.dma_start(out=c_sel2q[:], in_=sel2q.rearrange("h (m p) -> h m p", m=4))
        c_sel2k = const.tile([2, 128], BF16)
        nc.sync.dma_start(out=c_sel2k[:], in_=sel2k)
        c_id = const.tile([128, 128], BF16)
        nc.sync.dma_start(out=c_id[:], in_=ident)

        # persistent attention inputs
        qhat = persist.tile([128, 4, T], BF16)
        khatA = persist.tile([128, T], BF16)
        khatB = persist.tile([128, T], BF16)
        vslab = persist.tile([128, NTT, 130], BF16)
        y_T = persist.tile([128, 4, T], BF16)

        # ---------------- Phase A: projections + rope + norm ----------------
        with tc.tile_pool(name="xphase", bufs=1) as xp, \
             tc.tile_pool(name="rope", bufs=2) as rope, \
             tc.tile_pool(name="rq", bufs=5) as rqp, \
             tc.tile_pool(name="sq", bufs=5) as sqp, \
             tc.tile_pool(name="small", bufs=2) as small, \
             tc.tile_pool(name="qk_ps", bufs=2, space="PSUM") as qk_ps, \
             tc.tile_pool(name="nm_ps", bufs=1, space="PSUM") as nm_ps, \
             tc.tile_pool(name="bc_ps", bufs=2, space="PSUM") as bc_ps, \
             tc.tile_pool(name="v_ps", bufs=2, space="PSUM") as v_ps, \
             tc.tile_pool(name="vt_ps", bufs=1, space="PSUM") as vt_ps:

            c_C = xp.tile([128, T], F32)
            c_S = xp.tile([128, T], F32)
            x_sb = xp.tile([128, ND, T], BF16)
            wq_sb = xp.tile([128, ND, 512], BF16)
            wk_sb = xp.tile([128, ND, 128], BF16)
            wv_sb = xp.tile([128, ND, 128], BF16)
            for d in range(ND):
                nc.sync.dma_start(out=x_sb[:, d, :], in_=xT[128 * d:128 * (d + 1), :])
                nc.scalar.dma_start(out=wq_sb[:, d, :], in_=wqT[128 * d:128 * (d + 1), :])
                nc.scalar.dma_start(out=wk_sb[:, d, :], in_=wkT[128 * d:128 * (d + 1), :])
                nc.scalar.dma_start(out=wv_sb[:, d, :], in_=wvT[128 * d:128 * (d + 1), :])
                if d == 1:
                    nc.scalar.dma_start(out=c_C[:], in_=Ct)
                    nc.scalar.dma_start(out=c_S[:], in_=Sp)

            nc.vector.memset(vslab[:, :, 64:65], 1.0)
            nc.vector.memset(vslab[:, :, 129:130], 1.0)

            for c in range(NT):
                cs = slice(512 * c, 512 * (c + 1))
                sq_tiles = []
                rq_tiles = []
                for m in range(4):
                    qp = qk_ps.tile([128, 512], F32, tag="qkps")
                    for d in range(ND):
                        nc.tensor.matmul(qp[:], wq_sb[:, d, 128 * m:128 * (m + 1)],
                                         x_sb[:, d, cs], start=(d == 0), stop=(d == ND - 1))
                    qsw = rope.tile([128, 512], F32, tag="qsw")
                    u = rope.tile([128, 512], F32, tag="u")
                    with tc.high_priority(offset=120):
                        nc.vector.stream_shuffle(out=qsw[:], in_=qp[:], mask=SHUF)
                        nc.vector.tensor_tensor(out=u[:], in0=qp[:], in1=c_C[:, cs], op=MUL)
                    sqt = sqp.tile([128, 512], BF16, tag="sq")
                    nc.scalar.activation(out=sqt[:], in_=qp[:], func=SQUARE)
                    t = rope.tile([128, 512], F32, tag="t")
                    nc.vector.tensor_tensor(out=t[:], in0=qsw[:], in1=c_S[:, cs], op=MUL)
                    rq = rqp.tile([128, 512], F32, tag="rq")
                    nc.vector.tensor_tensor(out=rq[:], in0=u[:], in1=t[:], op=ADD)
                    sq_tiles.append(sqt)
                    rq_tiles.append(rq)
                nm = nm_ps.tile([8, 512], F32, tag="nm")
                for m in range(4):
                    nc.tensor.matmul(nm[:], c_selq[:, m, :], sq_tiles[m][:],
                                     start=(m == 0), stop=(m == 3))
                rn = small.tile([8, 512], F32, tag="rn")
                nc.vector.reciprocal(out=rn[:], in_=nm[:])
                fac = small.tile([8, 512], BF16, tag="fac")
                nc.scalar.activation(out=fac[:], in_=rn[:], func=SQRT, scale=c_fsq[:])
                for m in range(4):
                    bc = bc_ps.tile([128, 512], F32, tag="bcps")
                    nc.tensor.matmul(bc[:], c_sel2q[:, m, :], fac[:], start=True, stop=True)
                    nc.vector.tensor_tensor(out=qhat[:, m, cs], in0=rq_tiles[m][:],
                                            in1=bc[:], op=MUL)

                # K (one m-tile: 2 kv heads)
                kp = qk_ps.tile([128, 512], F32, tag="qkps")
                for d in range(ND):
                    nc.tensor.matmul(kp[:], wk_sb[:, d, :], x_sb[:, d, cs],
                                     start=(d == 0), stop=(d == ND - 1))
                ksw = rope.tile([128, 512], F32, tag="qsw")
                nc.vector.stream_shuffle(out=ksw[:], in_=kp[:], mask=SHUF)
                uk = rope.tile([128, 512], F32, tag="u")
                nc.vector.tensor_tensor(out=uk[:], in0=kp[:], in1=c_C[:, cs], op=MUL)
                sqk = sqp.tile([128, 512], BF16, tag="sq")
                nc.scalar.activation(out=sqk[:], in_=kp[:], func=SQUARE)
                tk = rope.tile([128, 512], F32, tag="t")
                nc.vector.tensor_tensor(out=tk[:], in0=ksw[:], in1=c_S[:, cs], op=MUL)
                rk = rqp.tile([128, 512], F32, tag="rq")
                nc.vector.tensor_tensor(out=rk[:], in0=uk[:], in1=tk[:], op=ADD)
                nmk = nm_ps.tile([2, 512], F32, tag="nm")
                nc.tensor.matmul(nmk[:], c_selk[:], sqk[:], start=True, stop=True)
                rnk = small.tile([2, 512], F32, tag="rnk")
                nc.vector.reciprocal(out=rnk[:], in_=nmk[:])
                fack = small.tile([2, 512], BF16, tag="fack")
                nc.scalar.activation(out=fack[:], in_=rnk[:], func=SQRT)
                bck = bc_ps.tile([128, 512], F32, tag="bcps")
                nc.tensor.matmul(bck[:], c_sel2k[:], fack[:], start=True, stop=True)
                nc.vector.tensor_tensor(out=khatA[0:64, cs], in0=rk[0:64, :],
                                        in1=bck[0:64, :], op=MUL)
                nc.vector.tensor_tensor(out=khatB[64:128, cs], in0=rk[64:128, :],
                                        in1=bck[64:128, :], op=MUL)
                # duplicate k heads so scores operands share base partitions
                nc.sync.dma_start(out=khatA[64:128, cs], in_=khatA[0:64, cs])
                nc.sync.dma_start(out=khatB[0:64, cs], in_=khatB[64:128, cs])

                # V projection (feature-major), then PE-transpose to token-major
                vf = v_ps.tile([128, 512], F32, tag="vps")
                for d in range(ND):
                    nc.tensor.matmul(vf[:], wv_sb[:, d, :], x_sb[:, d, cs],
                                     start=(d == 0), stop=(d == ND - 1))
                vfs = rope.tile([128, 512], BF16, tag="vfs")
                nc.vector.tensor_copy(out=vfs[:], in_=vf[:])
                for q4 in range(4):
                    tt = 4 * c + q4
                    vt = vt_ps.tile([128, 128], BF16, tag="vtps")
                    nc.tensor.transpose(vt[:], vfs[:, 128 * q4:128 * (q4 + 1)], c_id[:])
                    nc.vector.tensor_copy(out=vslab[:, tt, 0:64], in_=vt[:, 0:64])
                    nc.vector.tensor_copy(out=vslab[:, tt, 65:129], in_=vt[:, 64:128])

        # ---------------- Phase B: attention + output projection ----------------
        with tc.tile_pool(name="pT", bufs=3) as pTp, \
             tc.tile_pool(name="att_small", bufs=6) as asm, \
             tc.tile_pool(name="ystage", bufs=4) as ystg, \
             tc.tile_pool(name="wo", bufs=1) as wop, \
             tc.tile_pool(name="ostage", bufs=4) as ostg, \
             tc.tile_pool(name="dram", bufs=8, space="DRAM") as drp, \
             tc.tile_pool(name="s_ps", bufs=2, space="PSUM") as s_ps, \
             tc.tile_pool(name="y_ps", bufs=2, space="PSUM") as y_ps, \
             tc.tile_pool(name="o_ps", bufs=2, space="PSUM") as o_ps:

            wo_sb = wop.tile([128, 4, DIM], BF16)
            for yt in range(4):
                nc.sync.dma_start(out=wo_sb[:, yt, :], in_=woT[128 * yt:128 * (yt + 1), :])

            # head pairs (2*hp, 2*hp+1): same kv group, q/k at row groups
            # 0-63 / 64-127 -> the two score matmuls run concurrently on PE.
            # Chunk-outer order so o-proj of chunk c overlaps attention c+1.
            for c in range(NT):
                jmax = 4 * c + 3
                cs = slice(512 * c, 512 * (c + 1))
                for hp in range(4):
                    kd = khatA if hp < 2 else khatB
                    m = hp
                    vc = slice(0, 65) if hp < 2 else slice(65, 130)
                    pT = pTp.tile([128, NTT, 2, 512], BF16, tag="pT")
                    # zero the left margins of the diagonal blocks (both heads)
                    nc.gpsimd.memset(pT[:, 4 * c + 1:4 * c + 4, :, :], 0.0)
                    for j in range(jmax + 1):
                        o = max(0, 128 * j - 512 * c)
                        sps = s_ps.tile([128, 2, 512], F32, tag="sps")
                        for hi in (0, 1):
                            b = 64 * hi
                            nc.tensor.matmul(
                                sps[:, hi, o:512],
                                kd[b:b + 64, 128 * j:128 * (j + 1)],
                                qhat[b:b + 64, m, 512 * c + o: 512 * (c + 1)],
                                start=True, stop=True)
                        nc.scalar.activation(out=pT[:, j, :, o:512],
                                             in_=sps[:, :, o:512], func=EXP)
                        if 128 * j >= 512 * c:  # diagonal block: causal 0/1 mask
                            mb2 = bass.AP(tensor=c_m01.tensor, offset=c_m01[:].offset,
                                          ap=[list(c_m01[:].ap[0]), [0, 2],
                                              list(c_m01[:].ap[1])])
                            nc.gpsimd.tensor_tensor(out=pT[:, j, :, o:o + 128],
                                                    in0=pT[:, j, :, o:o + 128],
                                                    in1=mb2, op=MUL)
                    for hi in (0, 1):
                        yp = y_ps.tile([128, 512], F32, tag="yps")
                        for j in range(jmax + 1):
                            nc.tensor.matmul(yp[0:65, :], vslab[:, j, vc],
                                             pT[:, j, hi, :],
                                             start=(j == 0), stop=(j == jmax))
                        rden = asm.tile([1, 512], F32, tag="rden")
                        nc.vector.reciprocal(out=rden[:], in_=yp[64:65, :])
                        db = drp.tile([1, 512], F32, tag="db")
                        nc.sync.dma_start(out=db[:], in_=rden[:])
                        rdb = asm.tile([64, 512], F32, tag="rdb")
                        src = bass.AP(tensor=db.tensor, offset=db[:].offset,
                                      ap=[[0, 64]] + list(db[:].ap[1:]))
                        nc.sync.dma_start(out=rdb[:], in_=src)
                        if hi == 0:
                            nc.vector.tensor_tensor(out=y_T[0:64, m, cs],
                                                    in0=yp[0:64, :], in1=rdb[:],
                                                    op=MUL)
                        else:
                            yst = ystg.tile([64, 512], BF16, tag="yst")
                            nc.vector.tensor_tensor(out=yst[:], in0=yp[0:64, :],
                                                    in1=rdb[:], op=MUL)
                            nc.scalar.dma_start(out=y_T[64:128, m, cs], in_=yst[:])

                # output projection for this token chunk
                for mo in range(ND):
                    op = o_ps.tile([128, 512], F32, tag="ops")
                    for yt in range(4):
                        nc.tensor.matmul(op[:], wo_sb[:, yt, 128 * mo:128 * (mo + 1)],
                                         y_T[:, yt, cs], start=(yt == 0), stop=(yt == 3))
                    ost = ostg.tile([128, 512], F32, tag="ost")
                    nc.any.tensor_copy(out=ost[:], in_=op[:])
                    nc.scalar.dma_start(out=outT[128 * mo:128 * (mo + 1), cs], in_=ost[:])
    return nc


def postprocess(results, B=2, T=2048, DIM=2048):
    out = np.empty((B, T, DIM), np.float32)
    for b in range(B):
        acc = results[4 * b]["outT"].astype(np.float32)
        for i in range(1, 4):
            acc = acc + results[4 * b + i]["outT"]
        out[b] = acc.T
    return out


# ------------- multi-wait splitting (neuronxcc single-wait limit) -------------
import concourse.mybir as mybir

def split_multi_waits(nc):
    for f in nc.m.functions:
        for blk in f.blocks:
            insts = list(blk.instructions)
            changed = False
            out = []
            for inst in insts:
                si = getattr(inst, "sync_info", None)
                if si is not None and len(si.on_wait) > 1:
                    waits = list(si.on_wait)
                    for j, w in enumerate(waits[:-1]):
                        d = mybir.InstDrain(name=f"{inst.name}-sw{j}", ins=[], outs=[])
                        d.engine = inst.engine
                        d.sync_info = mybir.SyncInfo(on_wait=[w], on_update=[])
                        out.append(d)
                    inst.sync_info = mybir.SyncInfo(
                        on_wait=[waits[-1]], on_update=list(si.on_update)
                    )
                    changed = True
                out.append(inst)
            if changed:
                blk.instructions = out


# ---------------------------------------------------------------- entry point
_CACHE = {}


def kernel(x, freqs_cos, freqs_sin, wq, wk, wv, wo, q_scale, k_scale):
    """Full-input GQA attention on 8 NeuronCores; returns [2, 2048, 2048] f32."""
    from concourse.bass_utils import run_bass_kernel_spmd

    x = np.asarray(x, dtype=np.float32)
    freqs_cos = np.asarray(freqs_cos, dtype=np.float32)
    freqs_sin = np.asarray(freqs_sin, dtype=np.float32)
    wq = np.asarray(wq, dtype=np.float32)
    wk = np.asarray(wk, dtype=np.float32)
    wv = np.asarray(wv, dtype=np.float32)
    wo = np.asarray(wo, dtype=np.float32)

    if "nc" not in _CACHE:
        nc = build_nc(T=2048, DIM=2048)
        split_multi_waits(nc)
        _CACHE["nc"] = nc
    nc = _CACHE["nc"]

    in_maps = prep_core_inputs(x, freqs_cos, freqs_sin, wq, wk, wv, wo,
                               q_scale, k_scale, T=2048, DIM=2048)
    res = run_bass_kernel_spmd(nc, in_maps, core_ids=list(range(8)))
    return postprocess(res.results)



# revision 2
# speedup vs baseline: 1.0203x; 1.0203x over previous
"""Trainium2 Bass kernel v2 for GQA attention (RoPE + QK-L2norm + causal + wo).

Full-problem shapes: B=2, T=2048, DIM=2048, H=32 q-heads, G=8 kv-heads, D=64.
Sharding over 8 cores: core = (batch b = c//4, kv-pair p = c%4) ->
8 q heads / 2 kv heads per core. Each core computes a partial token-major
output out[T, DIM] (bf16); host sums 4 partials per batch (no transpose).

v2 vs baseline:
- fused chunk pipeline (proj c+1 / attention c / o-proj c-1 interleaved in the
  PE stream so PE never waits on the Act-engine softmax)
- attnV in orientation B (out [tok, d], contraction over kv): ~2x fewer PE
  cycles than feature-major orientation; diagonal blocks trimmed
- V projected directly token-major (x as the stationary operand): no V
  transposes
- y transposed to feature-major via DMA xbar transpose (free on engines)
- division by the softmax denominator folded into the y psum->sbuf copy
- all DMAs issued from SP; output staged bf16
"""
import math
from collections import deque
from contextlib import ExitStack

import numpy as np
import ml_dtypes

import concourse.bass as bass
import concourse.tile as tile
from concourse import mybir

F32 = mybir.dt.float32
BF16 = mybir.dt.bfloat16

MUL = mybir.AluOpType.mult
ADD = mybir.AluOpType.add
EXP = mybir.ActivationFunctionType.Exp
SQRT = mybir.ActivationFunctionType.Sqrt
SHUF = [(i + 16) % 32 for i in range(32)]


# ---------------------------------------------------------------- host prep
def _head_perm(D=64):
    """Permutation: new position j (0..63) -> original dim index (rope pairs
    at partition offset +-16 inside each 32-block)."""
    perm = np.empty(D, dtype=np.int64)
    for j in range(D):
        block, q = j // 32, j % 32
        freq = (q % 16) + 16 * block
        perm[j] = 2 * freq + (1 if q >= 16 else 0)
    return perm


def _freq_of_partition(P=128):
    fr = np.empty(P, dtype=np.int64)
    sign = np.empty(P, dtype=np.float32)
    for p in range(P):
        fr[p] = (p % 16) + 16 * ((p % 64) // 32)
        sign[p] = -1.0 if (p % 32) < 16 else 1.0
    return fr, sign


def prep_core_inputs(x, freqs_cos, freqs_sin, wq, wk, wv, wo, q_scale, k_scale,
                     T=2048, DIM=2048, D=64):
    bf = ml_dtypes.bfloat16
    perm = _head_perm(D)
    fr, sign = _freq_of_partition(128)

    C = freqs_cos[:, fr].T.astype(np.float32).copy()          # [128, T]
    Sp = (sign[:, None] * freqs_sin[:, fr].T).astype(np.float32).copy()

    r, c = np.mgrid[0:128, 0:128]
    mask01 = np.where(c >= r, 1.0, 0.0).astype(bf)

    selq = np.zeros((128, 4, 8), np.float32)
    for m in range(4):
        selq[0:64, m, 2 * m] = 1.0
        selq[64:128, m, 2 * m + 1] = 1.0
    selq = selq.reshape(128, 32).astype(bf).copy()
    selk = np.zeros((128, 2), bf)
    selk[0:64, 0] = 1.0
    selk[64:128, 1] = 1.0
    sel2q = np.zeros((8, 4, 128), np.float32)
    for m in range(4):
        sel2q[2 * m, m, 0:64] = 1.0
        sel2q[2 * m + 1, m, 64:128] = 1.0
    sel2q = sel2q.reshape(8, 512).astype(bf).copy()
    sel2k = np.zeros((2, 128), bf)
    sel2k[0, 0:64] = 1.0
    sel2k[1, 64:128] = 1.0

    qs = np.asarray(q_scale).reshape(-1)   # [32]
    ks = np.asarray(k_scale).reshape(-1)   # [8]

    xT = {b: np.ascontiguousarray(x[b].T).astype(bf) for b in range(x.shape[0])}

    in_maps = []
    for core in range(8):
        b, p = core // 4, core % 4
        wq_rows = wq[512 * p: 512 * (p + 1), :].reshape(8, D, DIM)[:, perm, :]
        wqT = np.ascontiguousarray(wq_rows.reshape(512, DIM).T).astype(bf)
        wk_rows = wk[128 * p: 128 * (p + 1), :].reshape(2, D, DIM)[:, perm, :]
        wkT = np.ascontiguousarray(wk_rows.reshape(128, DIM).T).astype(bf)
        wvT = np.ascontiguousarray(wv[128 * p: 128 * (p + 1), :].T).astype(bf)
        woT = np.ascontiguousarray(wo[:, 512 * p: 512 * (p + 1)].T).astype(bf)

        fh = np.empty((8, 1), np.float32)
        for h in range(8):
            fh[h, 0] = (qs[8 * p + h] * ks[2 * p + h // 4] / math.sqrt(D)) ** 2

        in_maps.append({
            "xT": xT[b], "wqT": wqT, "wkT": wkT, "wvT": wvT, "woT": woT,
            "Ct": C, "Sp": Sp, "mask01": mask01, "fsq": fh,
            "selq": selq, "selk": selk, "sel2q": sel2q, "sel2k": sel2k,
            "ident": np.eye(128, dtype=bf),
        })
    return in_maps


# ---------------------------------------------------------------- device code
def build_nc(T=2048, DIM=2048):
    NT = T // 512          # token chunks
    ND = DIM // 128        # contraction tiles
    NTT = T // 128         # 128-token tiles
    nc = bass.Bass("TRN2", target_bir_lowering=False, debug=False)

    xT = nc.dram_tensor("xT", [DIM, T], BF16, kind="ExternalInput").ap()
    wqT = nc.dram_tensor("wqT", [DIM, 512], BF16, kind="ExternalInput").ap()
    wkT = nc.dram_tensor("wkT", [DIM, 128], BF16, kind="ExternalInput").ap()
    wvT = nc.dram_tensor("wvT", [DIM, 128], BF16, kind="ExternalInput").ap()
    woT = nc.dram_tensor("woT", [512, DIM], BF16, kind="ExternalInput").ap()
    Ct = nc.dram_tensor("Ct", [128, T], F32, kind="ExternalInput").ap()
    Sp = nc.dram_tensor("Sp", [128, T], F32, kind="ExternalInput").ap()
    mask01 = nc.dram_tensor("mask01", [128, 128], BF16, kind="ExternalInput").ap()
    fsq = nc.dram_tensor("fsq", [8, 1], F32, kind="ExternalInput").ap()
    selq = nc.dram_tensor("selq", [128, 32], BF16, kind="ExternalInput").ap()
    selk = nc.dram_tensor("selk", [128, 2], BF16, kind="ExternalInput").ap()
    sel2q = nc.dram_tensor("sel2q", [8, 512], BF16, kind="ExternalInput").ap()
    sel2k = nc.dram_tensor("sel2k", [2, 128], BF16, kind="ExternalInput").ap()
    ident = nc.dram_tensor("ident", [128, 128], BF16, kind="ExternalInput").ap()
    outT = nc.dram_tensor("outT", [T, DIM], BF16, kind="ExternalOutput").ap()

    with tile.TileContext(nc) as tc, ExitStack() as ctx:
        const = ctx.enter_context(tc.tile_pool(name="const", bufs=1))
        wpool = ctx.enter_context(tc.tile_pool(name="wpool", bufs=1))
        persist = ctx.enter_context(tc.tile_pool(name="persist", bufs=1))
        xpool = ctx.enter_context(tc.tile_pool(name="xp", bufs=2))
        csp = ctx.enter_context(tc.tile_pool(name="csp", bufs=2))
        qhp = ctx.enter_context(tc.tile_pool(name="qh", bufs=2))
        rope = ctx.enter_context(tc.tile_pool(name="rope", bufs=6))
        sqp = ctx.enter_context(tc.tile_pool(name="sq", bufs=5))
        small = ctx.enter_context(tc.tile_pool(name="small", bufs=6))
        pTp = ctx.enter_context(tc.tile_pool(name="pT", bufs=2))
        ysp = ctx.enter_context(tc.tile_pool(name="ys", bufs=3))
        yTp = ctx.enter_context(tc.tile_pool(name="yT", bufs=1))
        ostp = ctx.enter_context(tc.tile_pool(name="ost", bufs=2))
        pp = ctx.enter_context(tc.tile_pool(name="pp", bufs=2, space="PSUM"))
        spp = ctx.enter_context(tc.tile_pool(name="sp", bufs=2, space="PSUM"))
        ypp = ctx.enter_context(tc.tile_pool(name="yp", bufs=2, space="PSUM"))

        # ---------------- constants + weights ----------------
        # DMA issue order matters: everything serializes on HWDGE + the DMA
        # engines, and the first projection matmuls need x(0) + wq only.
        wq_sb = wpool.tile([128, ND, 512], BF16)
        wk_sb = wpool.tile([128, ND, 128], BF16)
        wv_sb = wpool.tile([128, ND, 128], BF16)
        wo_sb = wpool.tile([128, 4, DIM], BF16)
        c_m01 = const.tile([128, 128], BF16)
        c_fsq = const.tile([8, 1], F32)
        c_selq = const.tile([128, 4, 8], BF16)
        c_selk = const.tile([128, 2], BF16)
        c_sel2q = const.tile([8, 4, 128], BF16)
        c_sel2k = const.tile([2, 128], BF16)
        c_id = const.tile([128, 128], BF16)

        def load_consts_mid():
            nc.sync.dma_start(out=c_fsq[:], in_=fsq)
            nc.sync.dma_start(out=c_selq[:], in_=selq.rearrange("p (m h) -> p m h", m=4))
            nc.sync.dma_start(out=c_sel2q[:], in_=sel2q.rearrange("h (m p) -> h m p", m=4))
            nc.sync.dma_start(out=wk_sb[:], in_=wkT.rearrange("(d p) f -> p d f", p=128))
            nc.sync.dma_start(out=c_selk[:], in_=selk)
            nc.sync.dma_start(out=c_sel2k[:], in_=sel2k)
            nc.sync.dma_start(out=c_id[:], in_=ident)
            nc.sync.dma_start(out=wv_sb[:], in_=wvT.rearrange("(d p) f -> p d f", p=128))
            nc.sync.dma_start(out=c_m01[:], in_=mask01)

        def load_consts_late():
            nc.sync.dma_start(out=wo_sb[:], in_=woT.rearrange("(f p) m -> p f m", p=128))


        khatA = persist.tile([128, T], BF16)
        khatB = persist.tile([128, T], BF16)
        vslab = persist.tile([128, NTT, 130], BF16)
        nc.vector.memset(vslab[:, :, 64:65], 1.0)
        nc.vector.memset(vslab[:, :, 129:130], 1.0)

        x_sb = {}     # chunk -> tile
        cC = {}
        cS = {}
        qhat = {}     # chunk -> tile
        yT_sb = {}    # chunk -> tile

        def load_x(c):
            x_sb[c] = xpool.tile([128, ND, 512], BF16, tag="x", name=f"x{c}")
            for h in (0, 1):
                nc.sync.dma_start(
                    out=x_sb[c][:, 8 * h:8 * (h + 1), :],
                    in_=xT.rearrange("(d p) t -> p d t", p=128)
                        [:, 8 * h:8 * (h + 1), 512 * c:512 * (c + 1)])
            cC[c] = csp.tile([128, 512], F32, tag="C", name=f"C{c}")
            nc.sync.dma_start(out=cC[c][:], in_=Ct[:, 512 * c:512 * (c + 1)])
            cS[c] = csp.tile([128, 512], F32, tag="S", name=f"S{c}")
            nc.sync.dma_start(out=cS[c][:], in_=Sp[:, 512 * c:512 * (c + 1)])

        # ---------------- projection emission units ----------------
        def proj_units(c):
            """Yield closures; each emits ~0.4-1.7us of PE work + its
            side-engine ops. Ordering within the list is the PE program
            order for chunk c's projections."""
            cs = slice(512 * c, 512 * (c + 1))
            units = []
            sq_tiles = []
            rq_tiles = []

            def qtile(m):
                def emit_mm(qp, d0):
                    for d in range(d0, d0 + 8):
                        nc.tensor.matmul(qp[:], wq_sb[:, d, 128 * m:128 * (m + 1)],
                                         x_sb[c][:, d, :], start=(d == 0),
                                         stop=(d == ND - 1))
                qp_box = {}

                def u1():
                    qp_box['t'] = pp.tile([128, 512], F32, tag="pp", name="qp")
                    emit_mm(qp_box['t'], 0)

                def u2():
                    qp = qp_box['t']
                    emit_mm(qp, 8)
                    qsw = rope.tile([128, 512], F32, tag="qsw", bufs=2)
                    nc.vector.stream_shuffle(out=qsw[:], in_=qp[:], mask=SHUF)
                    u = rope.tile([128, 512], F32, tag="u", bufs=2)
                    nc.vector.tensor_tensor(out=u[:], in0=qp[:], in1=cC[c][:], op=MUL)
                    t = rope.tile([128, 512], F32, tag="t", bufs=2)
                    nc.gpsimd.tensor_tensor(out=t[:], in0=qsw[:], in1=cS[c][:], op=MUL)
                    rq = rope.tile([128, 512], F32, tag="rq", bufs=5)
                    nc.gpsimd.tensor_tensor(out=rq[:], in0=u[:], in1=t[:], op=ADD)
                    # rope is norm-preserving: square the rotated (SBUF) copy
                    sqt = sqp.tile([128, 512], BF16, tag="sq")
                    nc.vector.tensor_tensor(out=sqt[:], in0=rq[:], in1=rq[:], op=MUL)
                    sq_tiles.append(sqt)
                    rq_tiles.append(rq)
                return [u1, u2]

            for m in range(4):
                units += qtile(m)

            def norm_q():
                nm = pp.tile([8, 512], F32, tag="pp")
                for m in range(4):
                    nc.tensor.matmul(nm[:], c_selq[:, m, :], sq_tiles[m][:],
                                     start=(m == 0), stop=(m == 3))
                rn = small.tile([8, 512], F32, tag="rn", bufs=1)
                fac = small.tile([8, 512], BF16, tag="fac", bufs=2)
                with tc.high_priority(offset=100):
                    nc.vector.reciprocal(out=rn[:], in_=nm[:])
                    nc.scalar.activation(out=fac[:], in_=rn[:], func=SQRT, scale=c_fsq[:])
                qhat[c] = qhp.tile([128, 4, 512], BF16, tag="qh", name=f"qh{c}")
                for m in range(4):
                    bc = pp.tile([128, 512], F32, tag="pp")
                    nc.tensor.matmul(bc[:], c_sel2q[:, m, :], fac[:], start=True, stop=True)
                    nc.vector.tensor_tensor(out=qhat[c][:, m, :], in0=rq_tiles[m][:],
                                            in1=bc[:], op=MUL)
            units.append(norm_q)

            kp_box = {}

            def k1():
                kp_box['t'] = pp.tile([128, 512], F32, tag="pp", name="kp")
                for d in range(8):
                    nc.tensor.matmul(kp_box['t'][:], wk_sb[:, d, :], x_sb[c][:, d, :],
                                     start=(d == 0), stop=False)

            def k2():
                kp = kp_box['t']
                for d in range(8, ND):
                    nc.tensor.matmul(kp[:], wk_sb[:, d, :], x_sb[c][:, d, :],
                                     start=False, stop=(d == ND - 1))
                ksw = rope.tile([128, 512], F32, tag="qsw", bufs=2)
                nc.vector.stream_shuffle(out=ksw[:], in_=kp[:], mask=SHUF)
                uk = rope.tile([128, 512], F32, tag="u", bufs=2)
                nc.vector.tensor_tensor(out=uk[:], in0=kp[:], in1=cC[c][:], op=MUL)
                tk = rope.tile([128, 512], F32, tag="t", bufs=2)
                nc.gpsimd.tensor_tensor(out=tk[:], in0=ksw[:], in1=cS[c][:], op=MUL)
                rk = rope.tile([128, 512], F32, tag="rq", bufs=5)
                nc.gpsimd.tensor_tensor(out=rk[:], in0=uk[:], in1=tk[:], op=ADD)
                sqk = sqp.tile([128, 512], BF16, tag="sq")
                nc.vector.tensor_tensor(out=sqk[:], in0=rk[:], in1=rk[:], op=MUL)
                kp_box['rk'] = rk
                kp_box['sqk'] = sqk

            def k3():
                nmk = pp.tile([2, 512], F32, tag="pp")
                nc.tensor.matmul(nmk[:], c_selk[:], kp_box['sqk'][:], start=True, stop=True)
                rnk = small.tile([2, 512], F32, tag="rnk", bufs=1)
                fack = small.tile([2, 512], BF16, tag="fack", bufs=2)
                with tc.high_priority(offset=100):
                    nc.vector.reciprocal(out=rnk[:], in_=nmk[:])
                    nc.scalar.activation(out=fack[:], in_=rnk[:], func=SQRT)
                bck = pp.tile([128, 512], F32, tag="pp")
                nc.tensor.matmul(bck[:], c_sel2k[:], fack[:], start=True, stop=True)
                rk = kp_box['rk']
                nc.vector.tensor_tensor(out=khatA[0:64, cs], in0=rk[0:64, :],
                                        in1=bck[0:64, :], op=MUL)
                nc.vector.tensor_tensor(out=khatB[64:128, cs], in0=rk[64:128, :],
                                        in1=bck[64:128, :], op=MUL)
                nc.sync.dma_start(out=khatA[64:128, cs], in_=khatA[0:64, cs])
                nc.sync.dma_start(out=khatB[0:64, cs], in_=khatB[64:128, cs])
            units_k12 = [k1, k2]
            unit_k3 = k3

            def vtile(u):
                def emit():
                    tt = 4 * c + u
                    vp = pp.tile([128, 128], F32, tag="pp")
                    for d in range(ND):
                        nc.tensor.matmul(vp[:], x_sb[c][:, d, 128 * u:128 * (u + 1)],
                                         wv_sb[:, d, :], start=(d == 0),
                                         stop=(d == ND - 1))
                    nc.vector.tensor_copy(out=vslab[:, tt, 0:64], in_=vp[:, 0:64])
                    nc.vector.tensor_copy(out=vslab[:, tt, 65:129], in_=vp[:, 64:128])
                return emit
            vts = [vtile(u) for u in range(4)]
            # norm chains (nm->recip->sqrt->bc and k's) span several engines;
            # keep psum-independent v-tiles between them so PE has filler
            nq = units.pop()        # norm_q emitted later
            units += units_k12
            units += [vts[0], vts[1], nq, vts[2], unit_k3, vts[3]]
            return units

        # ---------------- attention emission ----------------
        def emit_scores_j(c, hp, j):
            kd = khatA if hp < 2 else khatB
            o = max(0, 128 * j - 512 * c)
            s = spp.tile([128, 2, 512], F32, tag="sp")
            for hi in (0, 1):
                b = 64 * hi
                nc.tensor.matmul(
                    s[:, hi, o:512],
                    kd[b:b + 64, 128 * j:128 * (j + 1)],
                    qhat[c][b:b + 64, hp, o:512],
                    start=True, stop=True)
            nc.scalar.activation(out=pT_cur[0][:, j, :, o:512],
                                 in_=s[:, :, o:512], func=EXP)
            if 128 * j >= 512 * c:
                mb2 = bass.AP(tensor=c_m01.tensor, offset=c_m01[:].offset,
                              ap=[list(c_m01[:].ap[0]), [0, 2],
                                  list(c_m01[:].ap[1])])
                with tc.high_priority(offset=100):
                    nc.gpsimd.tensor_tensor(out=pT_cur[0][:, j, :, o:o + 128],
                                            in0=pT_cur[0][:, j, :, o:o + 128],
                                            in1=mb2, op=MUL)

        def av_units(c, hp, pT):
            vc = slice(0, 65) if hp < 2 else slice(65, 130)
            units = []

            def utile(u):
                jmax = 4 * c + u
                box = {}

                def mms():
                    yp = ypp.tile([128, 2, 65], F32, tag="yp")
                    for hi in (0, 1):
                        for j in range(jmax + 1):
                            nc.tensor.matmul(yp[:, hi, :],
                                             pT[:, j, hi, 128 * u:128 * (u + 1)],
                                             vslab[:, j, vc],
                                             start=(j == 0), stop=(j == jmax))
                    box['yp'] = yp

                def div():
                    yp = box['yp']
                    rden = small.tile([128, 2], F32, tag="rden", bufs=4)
                    ysb = ysp.tile([128, 128], BF16, tag="ys")
                    with tc.high_priority(offset=100):
                        nc.vector.reciprocal(out=rden[:], in_=yp[:, :, 64:65])
                        for hi in (0, 1):
                            nc.vector.tensor_scalar(
                                out=ysb[:, 64 * hi:64 * (hi + 1)], in0=yp[:, hi, 0:64],
                                scalar1=rden[:, hi:hi + 1], scalar2=None, op0=MUL)
                    tp = ypp.tile([128, 128], BF16, tag="yp", name="tp")
                    nc.tensor.transpose(tp[:], ysb[:], c_id[:])
                    dst = yT_sb[c][:, hp, 128 * u:128 * (u + 1)]
                    if (hp + u) % 2:
                        nc.scalar.copy(out=dst, in_=tp[:])
                    else:
                        nc.vector.tensor_copy(out=dst, in_=tp[:])
                return [mms, div]
            trip = [utile(u) for u in range(4)]
            # div of subtile u sits one attnV group behind: DVE gets headroom
            units += [trip[0][0], trip[1][0], trip[0][1], trip[2][0],
                      trip[1][1], trip[3][0], trip[2][1], trip[3][1]]
            return units

        def op_units(c, cycle_tags=False):
            units = []

            def optile(u, mo):
                def emit():
                    op = spp.tile([128, 512], F32, tag="sp", name="opx")
                    for ft in range(4):
                        nc.tensor.matmul(op[:], yT_sb[c][:, ft, 128 * u:128 * (u + 1)],
                                         wo_sb[:, ft, 512 * mo:512 * (mo + 1)],
                                         start=(ft == 0), stop=(ft == 3))
                    ost = ostp.tile([128, 512], BF16, tag="ost")
                    nc.vector.tensor_copy(out=ost[:], in_=op[:])
                    nc.sync.dma_start(
                        out=outT[512 * c + 128 * u:512 * c + 128 * (u + 1),
                                 512 * mo:512 * (mo + 1)],
                        in_=ost[:])
                return emit
            for u in range(4):
                for mo in range(4):
                    units.append(optile(u, mo))
            return units

        # ---------------- pipeline ----------------
        xr = xT.rearrange("(d p) t -> p d t", p=128)
        wqr = wqT.rearrange("(d p) f -> p d f", p=128)
        x_sb[0] = xpool.tile([128, ND, 512], BF16, tag="x", name="x0")
        nc.sync.dma_start(out=x_sb[0][:, 0:8, :], in_=xr[:, 0:8, 0:512])
        nc.sync.dma_start(out=wq_sb[:, :, 0:128], in_=wqr[:, :, 0:128])
        nc.sync.dma_start(out=wq_sb[:, :, 128:256], in_=wqr[:, :, 128:256])
        nc.sync.dma_start(out=x_sb[0][:, 8:16, :], in_=xr[:, 8:16, 0:512])
        cC[0] = csp.tile([128, 512], F32, tag="C", name="C0")
        nc.sync.dma_start(out=cC[0][:], in_=Ct[:, 0:512])
        cS[0] = csp.tile([128, 512], F32, tag="S", name="S0")
        nc.sync.dma_start(out=cS[0][:], in_=Sp[:, 0:512])
        for m in range(2, 4):
            nc.sync.dma_start(out=wq_sb[:, :, 128 * m:128 * (m + 1)],
                              in_=wqr[:, :, 128 * m:128 * (m + 1)])
        pT_cur = [None]

        pu0 = proj_units(0)
        pu0[0]()
        pu0[1]()
        load_consts_mid()
        for u in pu0[2:]:
            u()
        load_x(1)
        load_consts_late()

        fillers = deque()
        projq = deque()
        for c in range(NT):
            while projq:           # chunk c scores need qhat/khat/vslab of c
                projq.popleft()()
            yT_sb[c] = yTp.tile([128, 4, 512], BF16, tag="yT", name=f"yT{c}")
            if c == 1:
                load_x(2)
            if c == 2:
                load_x(3)
            if c < NT - 1:
                projq.extend(proj_units(c + 1))
            prev_av = None
            for hp in range(4):
                if prev_av:
                    for uu in prev_av:
                        fillers.appendleft(uu)
                    prev_av = None
                pT = pTp.tile([128, NTT, 2, 512], BF16, tag="pT")
                pT_cur[0] = pT
                for j in range(4 * c + 4):
                    emit_scores_j(c, hp, j)
                    for _ in range(2):
                        if fillers:
                            fillers.popleft()()
                    if projq:
                        projq.popleft()()
                prev_av = list(reversed(av_units(c, hp, pT)))
            if c < NT - 1:
                # interleave last head-pair's attnV with this chunk's o-proj
                # (op of subtile u goes a few units behind its transpose)
                avl = list(reversed(prev_av))
                opl = op_units(c)
                seq = avl[:4] + opl[0:2] + avl[4:6] + opl[2:6] + avl[6:] + opl[6:]
                for uu in reversed(seq):
                    fillers.appendleft(uu)
        # final chunk: interleave av(3,hp3) with o-proj per token subtile so
        # the o-proj psum (rotating through the now-idle score banks) chases
        # the last attention outputs
        avu = list(reversed(prev_av))
        opu = op_units(NT - 1, cycle_tags=True)
        # spread the 16 o-proj tiles behind the attnV units
        seq = avu[:5] + opu[0:2] + avu[5:7] + opu[2:6] + avu[7:] + opu[6:]
        for fn in seq:
            fn()
            if fillers:
                fillers.popleft()()
        while fillers:
            fillers.popleft()()
    return nc


def postprocess(results, B=2, T=2048, DIM=2048):
    out = np.empty((B, T, DIM), np.float32)
    for b in range(B):
        acc = results[4 * b]["outT"].astype(np.float32)
        for i in range(1, 4):
            acc = acc + results[4 * b + i]["outT"].astype(np.float32)
        out[b] = acc
    return out


# ------------- multi-wait splitting (neuronxcc single-wait limit) -------------
def split_multi_waits(nc):
    for f in nc.m.functions:
        for blk in f.blocks:
            insts = list(blk.instructions)
            changed = False
            out = []
            for inst in insts:
                si = getattr(inst, "sync_info", None)
                if si is not None and len(si.on_wait) > 1:
                    waits = list(si.on_wait)
                    for j, w in enumerate(waits[:-1]):
                        d = mybir.InstDrain(name=f"{inst.name}-sw{j}", ins=[], outs=[])
                        d.engine = inst.engine
                        d.sync_info = mybir.SyncInfo(on_wait=[w], on_update=[])
                        out.append(d)
                    inst.sync_info = mybir.SyncInfo(
                        on_wait=[waits[-1]], on_update=list(si.on_update)
                    )
                    changed = True
                out.append(inst)
            if changed:
                blk.instructions = out


# ---------------------------------------------------------------- entry point
_CACHE = {}


def kernel(x, freqs_cos, freqs_sin, wq, wk, wv, wo, q_scale, k_scale):
    """Full-input GQA attention on 8 NeuronCores; returns [2, 2048, 2048] f32."""
    from concourse.bass_utils import run_bass_kernel_spmd

    x = np.asarray(x, dtype=np.float32)
    freqs_cos = np.asarray(freqs_cos, dtype=np.float32)
    freqs_sin = np.asarray(freqs_sin, dtype=np.float32)
    wq = np.asarray(wq, dtype=np.float32)
    wk = np.asarray(wk, dtype=np.float32)
    wv = np.asarray(wv, dtype=np.float32)
    wo = np.asarray(wo, dtype=np.float32)

    if "nc" not in _CACHE:
        nc = build_nc(T=2048, DIM=2048)
        split_multi_waits(nc)
        _CACHE["nc"] = nc
    nc = _CACHE["nc"]

    in_maps = prep_core_inputs(x, freqs_cos, freqs_sin, wq, wk, wv, wo,
                               q_scale, k_scale, T=2048, DIM=2048)
    res = run_bass_kernel_spmd(nc, in_maps, core_ids=list(range(8)))
    return postprocess(res.results)


# revision 3
# speedup vs baseline: 1.0566x; 1.0356x over previous
"""Trainium2 Bass kernel v2 for GQA attention (RoPE + QK-L2norm + causal + wo).

Full-problem shapes: B=2, T=2048, DIM=2048, H=32 q-heads, G=8 kv-heads, D=64.
Sharding over 8 cores: core = (batch b = c//4, kv-pair p = c%4) ->
8 q heads / 2 kv heads per core. Each core computes a partial token-major
output out[T, DIM] (bf16); host sums 4 partials per batch (no transpose).

v2 vs baseline:
- fused chunk pipeline (proj c+1 / attention c / o-proj c-1 interleaved in the
  PE stream so PE never waits on the Act-engine softmax)
- attnV in orientation B (out [tok, d], contraction over kv): ~2x fewer PE
  cycles than feature-major orientation; diagonal blocks trimmed
- V projected directly token-major (x as the stationary operand): no V
  transposes
- y transposed to feature-major via DMA xbar transpose (free on engines)
- division by the softmax denominator folded into the y psum->sbuf copy
- all DMAs issued from SP; output staged bf16
"""
import math
from collections import deque
from contextlib import ExitStack

import numpy as np
import ml_dtypes

import concourse.bass as bass
import concourse.tile as tile
from concourse import mybir

F32 = mybir.dt.float32
BF16 = mybir.dt.bfloat16

MUL = mybir.AluOpType.mult
ADD = mybir.AluOpType.add
EXP = mybir.ActivationFunctionType.Exp
SQRT = mybir.ActivationFunctionType.Sqrt
SHUF = [(i + 16) % 32 for i in range(32)]


# ---------------------------------------------------------------- host prep
def _head_perm(D=64):
    """Permutation: new position j (0..63) -> original dim index (rope pairs
    at partition offset +-16 inside each 32-block)."""
    perm = np.empty(D, dtype=np.int64)
    for j in range(D):
        block, q = j // 32, j % 32
        freq = (q % 16) + 16 * block
        perm[j] = 2 * freq + (1 if q >= 16 else 0)
    return perm


def _freq_of_partition(P=128):
    fr = np.empty(P, dtype=np.int64)
    sign = np.empty(P, dtype=np.float32)
    for p in range(P):
        fr[p] = (p % 16) + 16 * ((p % 64) // 32)
        sign[p] = -1.0 if (p % 32) < 16 else 1.0
    return fr, sign


def prep_core_inputs(x, freqs_cos, freqs_sin, wq, wk, wv, wo, q_scale, k_scale,
                     T=2048, DIM=2048, D=64):
    bf = ml_dtypes.bfloat16
    perm = _head_perm(D)
    fr, sign = _freq_of_partition(128)

    C = freqs_cos[:, fr].T.astype(np.float32).copy()          # [128, T]
    Sp = (sign[:, None] * freqs_sin[:, fr].T).astype(np.float32).copy()

    r, c = np.mgrid[0:128, 0:128]
    mask01 = np.where(c >= r, 1.0, 0.0).astype(bf)

    selq = np.zeros((128, 4, 8), np.float32)
    for m in range(4):
        selq[0:64, m, 2 * m] = 1.0
        selq[64:128, m, 2 * m + 1] = 1.0
    selq = selq.reshape(128, 32).astype(bf).copy()
    selk = np.zeros((128, 2), bf)
    selk[0:64, 0] = 1.0
    selk[64:128, 1] = 1.0
    sel2q = np.zeros((8, 4, 128), np.float32)
    for m in range(4):
        sel2q[2 * m, m, 0:64] = 1.0
        sel2q[2 * m + 1, m, 64:128] = 1.0
    sel2q = sel2q.reshape(8, 512).astype(bf).copy()
    sel2k = np.zeros((2, 128), bf)
    sel2k[0, 0:64] = 1.0
    sel2k[1, 64:128] = 1.0

    qs = np.asarray(q_scale).reshape(-1)   # [32]
    ks = np.asarray(k_scale).reshape(-1)   # [8]

    xT = {b: np.ascontiguousarray(x[b].T).astype(bf) for b in range(x.shape[0])}

    in_maps = []
    for core in range(8):
        b, p = core // 4, core % 4
        wq_rows = wq[512 * p: 512 * (p + 1), :].reshape(8, D, DIM)[:, perm, :]
        wqT = np.ascontiguousarray(wq_rows.reshape(512, DIM).T).astype(bf)
        wk_rows = wk[128 * p: 128 * (p + 1), :].reshape(2, D, DIM)[:, perm, :]
        wkT = np.ascontiguousarray(wk_rows.reshape(128, DIM).T).astype(bf)
        wvT = np.ascontiguousarray(wv[128 * p: 128 * (p + 1), :].T).astype(bf)
        woT = np.ascontiguousarray(wo[:, 512 * p: 512 * (p + 1)].T).astype(bf)

        fh = np.empty((8, 1), np.float32)
        for h in range(8):
            fh[h, 0] = (qs[8 * p + h] * ks[2 * p + h // 4] / math.sqrt(D)) ** 2

        in_maps.append({
            "xT": xT[b], "wqT": wqT, "wkT": wkT, "wvT": wvT, "woT": woT,
            "Ct": C, "Sp": Sp, "mask01": mask01, "fsq": fh,
            "selq": selq, "selk": selk, "sel2q": sel2q, "sel2k": sel2k,
            "ident": np.eye(128, dtype=bf),
        })
    return in_maps


# ---------------------------------------------------------------- device code
def build_nc(T=2048, DIM=2048):
    NT = T // 512          # token chunks
    ND = DIM // 128        # contraction tiles
    NTT = T // 128         # 128-token tiles
    nc = bass.Bass("TRN2", target_bir_lowering=False, debug=False)

    xT = nc.dram_tensor("xT", [DIM, T], BF16, kind="ExternalInput").ap()
    wqT = nc.dram_tensor("wqT", [DIM, 512], BF16, kind="ExternalInput").ap()
    wkT = nc.dram_tensor("wkT", [DIM, 128], BF16, kind="ExternalInput").ap()
    wvT = nc.dram_tensor("wvT", [DIM, 128], BF16, kind="ExternalInput").ap()
    woT = nc.dram_tensor("woT", [512, DIM], BF16, kind="ExternalInput").ap()
    Ct = nc.dram_tensor("Ct", [128, T], F32, kind="ExternalInput").ap()
    Sp = nc.dram_tensor("Sp", [128, T], F32, kind="ExternalInput").ap()
    mask01 = nc.dram_tensor("mask01", [128, 128], BF16, kind="ExternalInput").ap()
    fsq = nc.dram_tensor("fsq", [8, 1], F32, kind="ExternalInput").ap()
    selq = nc.dram_tensor("selq", [128, 32], BF16, kind="ExternalInput").ap()
    selk = nc.dram_tensor("selk", [128, 2], BF16, kind="ExternalInput").ap()
    sel2q = nc.dram_tensor("sel2q", [8, 512], BF16, kind="ExternalInput").ap()
    sel2k = nc.dram_tensor("sel2k", [2, 128], BF16, kind="ExternalInput").ap()
    ident = nc.dram_tensor("ident", [128, 128], BF16, kind="ExternalInput").ap()
    outT = nc.dram_tensor("outT", [T, DIM], BF16, kind="ExternalOutput").ap()

    with tile.TileContext(nc) as tc, ExitStack() as ctx:
        const = ctx.enter_context(tc.tile_pool(name="const", bufs=1))
        wpool = ctx.enter_context(tc.tile_pool(name="wpool", bufs=1))
        persist = ctx.enter_context(tc.tile_pool(name="persist", bufs=1))
        xpool = ctx.enter_context(tc.tile_pool(name="xp", bufs=2))
        csp = ctx.enter_context(tc.tile_pool(name="csp", bufs=2))
        qhp = ctx.enter_context(tc.tile_pool(name="qh", bufs=2))
        rope = ctx.enter_context(tc.tile_pool(name="rope", bufs=6))
        sqp = ctx.enter_context(tc.tile_pool(name="sq", bufs=5))
        small = ctx.enter_context(tc.tile_pool(name="small", bufs=6))
        pTp = ctx.enter_context(tc.tile_pool(name="pT", bufs=2))
        ysp = ctx.enter_context(tc.tile_pool(name="ys", bufs=3))
        yTp = ctx.enter_context(tc.tile_pool(name="yT", bufs=1))
        ostp = ctx.enter_context(tc.tile_pool(name="ost", bufs=2))
        pp = ctx.enter_context(tc.tile_pool(name="pp", bufs=2, space="PSUM"))
        spp = ctx.enter_context(tc.tile_pool(name="sp", bufs=2, space="PSUM"))
        ypp = ctx.enter_context(tc.tile_pool(name="yp", bufs=2, space="PSUM"))

        # ---------------- constants + weights ----------------
        # DMA issue order matters: everything serializes on HWDGE + the DMA
        # engines, and the first projection matmuls need x(0) + wq only.
        wq_sb = wpool.tile([128, ND, 512], BF16)
        wk_sb = wpool.tile([128, ND, 128], BF16)
        wv_sb = wpool.tile([128, ND, 128], BF16)
        wo_sb = wpool.tile([128, 4, DIM], BF16)
        c_m01 = const.tile([128, 128], BF16)
        c_fsq = const.tile([8, 1], F32)
        c_selq = const.tile([128, 4, 8], BF16)
        c_selk = const.tile([128, 2], BF16)
        c_sel2q = const.tile([8, 4, 128], BF16)
        c_sel2k = const.tile([2, 128], BF16)
        c_id = const.tile([128, 128], BF16)

        def load_consts_mid():
            nc.sync.dma_start(out=c_fsq[:], in_=fsq)
            nc.sync.dma_start(out=c_selq[:], in_=selq.rearrange("p (m h) -> p m h", m=4))
            nc.sync.dma_start(out=c_sel2q[:], in_=sel2q.rearrange("h (m p) -> h m p", m=4))
            nc.sync.dma_start(out=wk_sb[:], in_=wkT.rearrange("(d p) f -> p d f", p=128))
            nc.sync.dma_start(out=wv_sb[:], in_=wvT.rearrange("(d p) f -> p d f", p=128))
            nc.sync.dma_start(out=c_selk[:], in_=selk)
            nc.sync.dma_start(out=c_sel2k[:], in_=sel2k)
            nc.sync.dma_start(out=c_id[:], in_=ident)
            nc.sync.dma_start(out=c_m01[:], in_=mask01)

        def load_consts_late():
            nc.sync.dma_start(out=wo_sb[:], in_=woT.rearrange("(f p) m -> p f m", p=128))


        khatA = persist.tile([128, T], BF16)
        khatB = persist.tile([128, T], BF16)
        vslab = persist.tile([128, NTT, 130], BF16)
        nc.vector.memset(vslab[:, :, 64:65], 1.0)
        nc.vector.memset(vslab[:, :, 129:130], 1.0)

        x_sb = {}     # chunk -> tile
        cC = {}
        cS = {}
        qhat = {}     # chunk -> tile
        yT_sb = {}    # chunk -> tile

        def load_x(c):
            x_sb[c] = xpool.tile([128, ND, 512], BF16, tag="x", name=f"x{c}")
            for h in (0, 1):
                nc.sync.dma_start(
                    out=x_sb[c][:, 8 * h:8 * (h + 1), :],
                    in_=xT.rearrange("(d p) t -> p d t", p=128)
                        [:, 8 * h:8 * (h + 1), 512 * c:512 * (c + 1)])
            cC[c] = csp.tile([128, 512], F32, tag="C", name=f"C{c}")
            nc.sync.dma_start(out=cC[c][:], in_=Ct[:, 512 * c:512 * (c + 1)])
            cS[c] = csp.tile([128, 512], F32, tag="S", name=f"S{c}")
            nc.sync.dma_start(out=cS[c][:], in_=Sp[:, 512 * c:512 * (c + 1)])

        # ---------------- projection emission units ----------------
        def proj_units(c):
            """Yield closures; each emits ~0.4-1.7us of PE work + its
            side-engine ops. Ordering within the list is the PE program
            order for chunk c's projections."""
            cs = slice(512 * c, 512 * (c + 1))
            units = []
            sq_tiles = []
            rq_tiles = []

            def qtile(m):
                def emit_mm(qp, d0):
                    for d in range(d0, d0 + 8):
                        nc.tensor.matmul(qp[:], wq_sb[:, d, 128 * m:128 * (m + 1)],
                                         x_sb[c][:, d, :], start=(d == 0),
                                         stop=(d == ND - 1))
                qp_box = {}

                def u1():
                    qp_box['t'] = pp.tile([128, 512], F32, tag="pp", name="qp")
                    emit_mm(qp_box['t'], 0)

                def u2():
                    qp = qp_box['t']
                    emit_mm(qp, 8)
                    qsw = rope.tile([128, 512], F32, tag="qsw", bufs=2)
                    nc.vector.stream_shuffle(out=qsw[:], in_=qp[:], mask=SHUF)
                    u = rope.tile([128, 512], F32, tag="u", bufs=2)
                    nc.vector.tensor_tensor(out=u[:], in0=qp[:], in1=cC[c][:], op=MUL)
                    t = rope.tile([128, 512], F32, tag="t", bufs=2)
                    nc.gpsimd.tensor_tensor(out=t[:], in0=qsw[:], in1=cS[c][:], op=MUL)
                    rq = rope.tile([128, 512], F32, tag="rq", bufs=5)
                    nc.gpsimd.tensor_tensor(out=rq[:], in0=u[:], in1=t[:], op=ADD)
                    # rope is norm-preserving: square the rotated (SBUF) copy
                    sqt = sqp.tile([128, 512], BF16, tag="sq")
                    nc.vector.tensor_tensor(out=sqt[:], in0=rq[:], in1=rq[:], op=MUL)
                    sq_tiles.append(sqt)
                    rq_tiles.append(rq)
                return [u1, u2]

            for m in range(4):
                units += qtile(m)

            def norm_q():
                nm = pp.tile([8, 512], F32, tag="pp")
                for m in range(4):
                    nc.tensor.matmul(nm[:], c_selq[:, m, :], sq_tiles[m][:],
                                     start=(m == 0), stop=(m == 3))
                rn = small.tile([8, 512], F32, tag="rn", bufs=1)
                fac = small.tile([8, 512], BF16, tag="fac", bufs=2)
                with tc.high_priority(offset=100):
                    nc.vector.reciprocal(out=rn[:], in_=nm[:])
                    nc.scalar.activation(out=fac[:], in_=rn[:], func=SQRT, scale=c_fsq[:])
                qhat[c] = qhp.tile([128, 4, 512], BF16, tag="qh", name=f"qh{c}")
                for m in range(4):
                    bc = pp.tile([128, 512], F32, tag="pp")
                    nc.tensor.matmul(bc[:], c_sel2q[:, m, :], fac[:], start=True, stop=True)
                    nc.vector.tensor_tensor(out=qhat[c][:, m, :], in0=rq_tiles[m][:],
                                            in1=bc[:], op=MUL)
            units.append(norm_q)

            kp_box = {}

            def k1():
                kp_box['t'] = pp.tile([128, 512], F32, tag="pp", name="kp")
                for d in range(8):
                    nc.tensor.matmul(kp_box['t'][:], wk_sb[:, d, :], x_sb[c][:, d, :],
                                     start=(d == 0), stop=False)

            def k2():
                kp = kp_box['t']
                for d in range(8, ND):
                    nc.tensor.matmul(kp[:], wk_sb[:, d, :], x_sb[c][:, d, :],
                                     start=False, stop=(d == ND - 1))
                ksw = rope.tile([128, 512], F32, tag="qsw", bufs=2)
                nc.vector.stream_shuffle(out=ksw[:], in_=kp[:], mask=SHUF)
                uk = rope.tile([128, 512], F32, tag="u", bufs=2)
                nc.vector.tensor_tensor(out=uk[:], in0=kp[:], in1=cC[c][:], op=MUL)
                tk = rope.tile([128, 512], F32, tag="t", bufs=2)
                nc.gpsimd.tensor_tensor(out=tk[:], in0=ksw[:], in1=cS[c][:], op=MUL)
                rk = rope.tile([128, 512], F32, tag="rq", bufs=5)
                nc.gpsimd.tensor_tensor(out=rk[:], in0=uk[:], in1=tk[:], op=ADD)
                sqk = sqp.tile([128, 512], BF16, tag="sq")
                nc.vector.tensor_tensor(out=sqk[:], in0=rk[:], in1=rk[:], op=MUL)
                kp_box['rk'] = rk
                kp_box['sqk'] = sqk

            def k3():
                nmk = pp.tile([2, 512], F32, tag="pp")
                nc.tensor.matmul(nmk[:], c_selk[:], kp_box['sqk'][:], start=True, stop=True)
                rnk = small.tile([2, 512], F32, tag="rnk", bufs=1)
                fack = small.tile([2, 512], BF16, tag="fack", bufs=2)
                with tc.high_priority(offset=100):
                    nc.vector.reciprocal(out=rnk[:], in_=nmk[:])
                    nc.scalar.activation(out=fack[:], in_=rnk[:], func=SQRT)
                bck = pp.tile([128, 512], F32, tag="pp")
                nc.tensor.matmul(bck[:], c_sel2k[:], fack[:], start=True, stop=True)
                rk = kp_box['rk']
                nc.vector.tensor_tensor(out=khatA[0:64, cs], in0=rk[0:64, :],
                                        in1=bck[0:64, :], op=MUL)
                nc.vector.tensor_tensor(out=khatB[64:128, cs], in0=rk[64:128, :],
                                        in1=bck[64:128, :], op=MUL)
                nc.sync.dma_start(out=khatA[64:128, cs], in_=khatA[0:64, cs])
                nc.sync.dma_start(out=khatB[0:64, cs], in_=khatB[64:128, cs])
            units_k12 = [k1, k2]
            unit_k3 = k3

            def vtile(u):
                def emit():
                    tt = 4 * c + u
                    vp = pp.tile([128, 128], F32, tag="pp")
                    for d in range(ND):
                        nc.tensor.matmul(vp[:], x_sb[c][:, d, 128 * u:128 * (u + 1)],
                                         wv_sb[:, d, :], start=(d == 0),
                                         stop=(d == ND - 1))
                    nc.vector.tensor_copy(out=vslab[:, tt, 0:64], in_=vp[:, 0:64])
                    nc.vector.tensor_copy(out=vslab[:, tt, 65:129], in_=vp[:, 64:128])
                return emit
            vts = [vtile(u) for u in range(4)]
            # norm chains (nm->recip->sqrt->bc and k's) span several engines;
            # keep psum-independent v-tiles between them so PE has filler
            nq = units.pop()        # norm_q emitted later
            units += units_k12
            units += [vts[0], vts[1], nq, vts[2], unit_k3, vts[3]]
            return units

        # ---------------- attention emission ----------------
        def emit_scores_j(c, hp, j):
            kd = khatA if hp < 2 else khatB
            o = max(0, 128 * j - 512 * c)
            s = spp.tile([128, 2, 512], F32, tag="sp")
            for hi in (0, 1):
                b = 64 * hi
                nc.tensor.matmul(
                    s[:, hi, o:512],
                    kd[b:b + 64, 128 * j:128 * (j + 1)],
                    qhat[c][b:b + 64, hp, o:512],
                    start=True, stop=True)
            nc.scalar.activation(out=pT_cur[0][:, j, :, o:512],
                                 in_=s[:, :, o:512], func=EXP)
            if 128 * j >= 512 * c:
                mb2 = bass.AP(tensor=c_m01.tensor, offset=c_m01[:].offset,
                              ap=[list(c_m01[:].ap[0]), [0, 2],
                                  list(c_m01[:].ap[1])])
                with tc.high_priority(offset=100):
                    nc.gpsimd.tensor_tensor(out=pT_cur[0][:, j, :, o:o + 128],
                                            in0=pT_cur[0][:, j, :, o:o + 128],
                                            in1=mb2, op=MUL)

        def av_units(c, hp, pT):
            vc = slice(0, 65) if hp < 2 else slice(65, 130)
            units = []

            def utile(u):
                jmax = 4 * c + u
                box = {}

                def mms():
                    yp = ypp.tile([128, 2, 65], F32, tag="yp")
                    for hi in (0, 1):
                        for j in range(jmax + 1):
                            nc.tensor.matmul(yp[:, hi, :],
                                             pT[:, j, hi, 128 * u:128 * (u + 1)],
                                             vslab[:, j, vc],
                                             start=(j == 0), stop=(j == jmax))
                    box['yp'] = yp

                def div():
                    yp = box['yp']
                    rden = small.tile([128, 2], F32, tag="rden", bufs=4)
                    ysb = ysp.tile([128, 128], BF16, tag="ys")
                    with tc.high_priority(offset=100):
                        nc.vector.reciprocal(out=rden[:], in_=yp[:, :, 64:65])
                        for hi in (0, 1):
                            nc.vector.tensor_scalar(
                                out=ysb[:, 64 * hi:64 * (hi + 1)], in0=yp[:, hi, 0:64],
                                scalar1=rden[:, hi:hi + 1], scalar2=None, op0=MUL)
                    tp = ypp.tile([128, 128], BF16, tag="yp", name="tp")
                    nc.tensor.transpose(tp[:], ysb[:], c_id[:])
                    dst = yT_sb[c][:, hp, 128 * u:128 * (u + 1)]
                    nc.vector.tensor_copy(out=dst, in_=tp[:])
                return [mms, div]
            trip = [utile(u) for u in range(4)]
            # div of subtile u sits one attnV group behind: DVE gets headroom
            units += [trip[0][0], trip[1][0], trip[0][1], trip[2][0],
                      trip[1][1], trip[3][0], trip[2][1], trip[3][1]]
            return units

        def op_units(c, cycle_tags=False):
            units = []

            def optile(u, mo):
                def emit():
                    if (u + mo) % 2:
                        op = pp.tile([128, 512], F32, tag="pp", name="opx")
                    else:
                        op = spp.tile([128, 512], F32, tag="sp", name="opx")
                    for ft in range(4):
                        nc.tensor.matmul(op[:], yT_sb[c][:, ft, 128 * u:128 * (u + 1)],
                                         wo_sb[:, ft, 512 * mo:512 * (mo + 1)],
                                         start=(ft == 0), stop=(ft == 3))
                    ost = ostp.tile([128, 512], BF16, tag="ost")
                    nc.vector.tensor_copy(out=ost[:], in_=op[:])
                    nc.sync.dma_start(
                        out=outT[512 * c + 128 * u:512 * c + 128 * (u + 1),
                                 512 * mo:512 * (mo + 1)],
                        in_=ost[:])
                return emit
            for u in range(4):
                for mo in range(4):
                    units.append(optile(u, mo))
            return units

        # ---------------- pipeline ----------------
        xr = xT.rearrange("(d p) t -> p d t", p=128)
        wqr = wqT.rearrange("(d p) f -> p d f", p=128)
        x_sb[0] = xpool.tile([128, ND, 512], BF16, tag="x", name="x0")
        nc.sync.dma_start(out=x_sb[0][:, 0:8, :], in_=xr[:, 0:8, 0:512])
        nc.sync.dma_start(out=wq_sb[:, :, 0:128], in_=wqr[:, :, 0:128])
        nc.sync.dma_start(out=wq_sb[:, :, 128:256], in_=wqr[:, :, 128:256])
        nc.sync.dma_start(out=x_sb[0][:, 8:16, :], in_=xr[:, 8:16, 0:512])
        cC[0] = csp.tile([128, 512], F32, tag="C", name="C0")
        nc.sync.dma_start(out=cC[0][:], in_=Ct[:, 0:512])
        cS[0] = csp.tile([128, 512], F32, tag="S", name="S0")
        nc.sync.dma_start(out=cS[0][:], in_=Sp[:, 0:512])
        for m in range(2, 4):
            nc.sync.dma_start(out=wq_sb[:, :, 128 * m:128 * (m + 1)],
                              in_=wqr[:, :, 128 * m:128 * (m + 1)])
        pT_cur = [None]

        pu0 = proj_units(0)
        pu0[0]()
        pu0[1]()
        load_consts_mid()
        for u in pu0[2:]:
            u()
        load_x(1)
        load_consts_late()

        fillers = deque()
        projq = deque()
        for c in range(NT):
            while projq:           # chunk c scores need qhat/khat/vslab of c
                projq.popleft()()
            yT_sb[c] = yTp.tile([128, 4, 512], BF16, tag="yT", name=f"yT{c}")
            if c == 1:
                load_x(2)
            if c == 2:
                load_x(3)
            if c < NT - 1:
                projq.extend(proj_units(c + 1))
            prev_av = None
            for hp in range(4):
                if prev_av:
                    for uu in prev_av:
                        fillers.appendleft(uu)
                    prev_av = None
                pT = pTp.tile([128, NTT, 2, 512], BF16, tag="pT")
                pT_cur[0] = pT
                for j in range(4 * c + 4):
                    emit_scores_j(c, hp, j)
                    for _ in range(2):
                        if fillers:
                            fillers.popleft()()
                    if projq:
                        projq.popleft()()
                prev_av = list(reversed(av_units(c, hp, pT)))
            if c < NT - 1:
                # interleave last head-pair's attnV with this chunk's o-proj
                # (op of subtile u goes a few units behind its transpose)
                avl = list(reversed(prev_av))
                opl = op_units(c)
                seq = avl[:4] + opl[0:2] + avl[4:6] + opl[2:6] + avl[6:] + opl[6:]
                for uu in reversed(seq):
                    fillers.appendleft(uu)
        # final chunk: interleave av(3,hp3) with o-proj per token subtile so
        # the o-proj psum (rotating through the now-idle score banks) chases
        # the last attention outputs
        avu = list(reversed(prev_av))
        opu = op_units(NT - 1, cycle_tags=True)
        # spread the 16 o-proj tiles behind the attnV units
        seq = avu[:5] + opu[0:2] + avu[5:7] + opu[2:6] + avu[7:] + opu[6:]
        for fn in seq:
            fn()
            if fillers:
                fillers.popleft()()
        while fillers:
            fillers.popleft()()
    return nc


def postprocess(results, B=2, T=2048, DIM=2048):
    out = np.empty((B, T, DIM), np.float32)
    for b in range(B):
        acc = results[4 * b]["outT"].astype(np.float32)
        for i in range(1, 4):
            acc = acc + results[4 * b + i]["outT"].astype(np.float32)
        out[b] = acc
    return out


# ------------- multi-wait splitting (neuronxcc single-wait limit) -------------
def split_multi_waits(nc):
    for f in nc.m.functions:
        for blk in f.blocks:
            insts = list(blk.instructions)
            changed = False
            out = []
            for inst in insts:
                si = getattr(inst, "sync_info", None)
                if si is not None and len(si.on_wait) > 1:
                    waits = list(si.on_wait)
                    for j, w in enumerate(waits[:-1]):
                        d = mybir.InstDrain(name=f"{inst.name}-sw{j}", ins=[], outs=[])
                        d.engine = inst.engine
                        d.sync_info = mybir.SyncInfo(on_wait=[w], on_update=[])
                        out.append(d)
                    inst.sync_info = mybir.SyncInfo(
                        on_wait=[waits[-1]], on_update=list(si.on_update)
                    )
                    changed = True
                out.append(inst)
            if changed:
                blk.instructions = out


# ---------------------------------------------------------------- entry point
_CACHE = {}


def kernel(x, freqs_cos, freqs_sin, wq, wk, wv, wo, q_scale, k_scale):
    """Full-input GQA attention on 8 NeuronCores; returns [2, 2048, 2048] f32."""
    from concourse.bass_utils import run_bass_kernel_spmd

    x = np.asarray(x, dtype=np.float32)
    freqs_cos = np.asarray(freqs_cos, dtype=np.float32)
    freqs_sin = np.asarray(freqs_sin, dtype=np.float32)
    wq = np.asarray(wq, dtype=np.float32)
    wk = np.asarray(wk, dtype=np.float32)
    wv = np.asarray(wv, dtype=np.float32)
    wo = np.asarray(wo, dtype=np.float32)

    if "nc" not in _CACHE:
        nc = build_nc(T=2048, DIM=2048)
        split_multi_waits(nc)
        _CACHE["nc"] = nc
    nc = _CACHE["nc"]

    in_maps = prep_core_inputs(x, freqs_cos, freqs_sin, wq, wk, wv, wo,
                               q_scale, k_scale, T=2048, DIM=2048)
    res = run_bass_kernel_spmd(nc, in_maps, core_ids=list(range(8)))
    return postprocess(res.results)
